# revision 12
# baseline (speedup 1.0000x reference)
"""DRConv (dynamic region-aware conv) Trainium2 kernel.

Math (per batch b, all on device):
  x_se  = 0.25*sigmoid(routing_w @ mean_hw(x) + routing_b)           # [G*T]
  Z_t   = conv3x3(x, template_t)       for t in 0..T-1               # [O, H, W]
  U     = [x_se.T | 1] contracted with exp(Alpha) over g             # [T+1, P]
  out   = (sum_t Z_t * U_t) / U_T  + bias                            # [O, H, W]
which equals the reference
  out = einsum('boghw,bghw->bohw', einsum('bokg,bkhw->boghw', w, patches),
               softmax(Alpha)) + bias
because w = blend(x_se, templates) commutes through the conv: the blend
weights x_se[g,t] and the softmax probs both act per (g, pixel), so the
G-sum and T-sum exchange with the K-contraction.

Sharding: data-parallel over batch B=8, one batch element per NeuronCore.
Templates/routing weights replicated. No collectives.

Device layout (per core):
  pixels live in a zero-padded 58x58 plane, flattened; pf = y*58 + x.
  conv = 9 shifted matmuls accumulating in PSUM:
    Z[px, (t,o)] += x[c, base+px+delta(i,j)].T @ tmpl[c, (t,o)]
  pixel tiles are the stationary operand (128 px per matmul), so the
  per-pixel softmax mixing becomes per-partition scalar_tensor_tensor ops,
  and the final [px, o] -> [o, px] flip is a PE transpose.
"""

import ml_dtypes
import numpy as np

import concourse.bass as bass
import concourse.mybir as mybir
from concourse import bacc, masks
from concourse.tile import TileContext
from concourse.bass_utils import run_bass_kernel_spmd

# problem constants
C = 128          # in channels
O = 128          # out channels
H = W = 56
G = 8            # groups
T = 8            # num weight templates
HP = WP = 58     # padded plane
NPIX = HP * WP   # 3364
GUARD = 64       # front guard in the x buffer for negative conv shifts
XFREE = 3584     # GUARD + NPIX rounded up (28*128)
OFREE = 3456     # 27*128, covers NPIX
PT0 = WP         # first pixel-tile starts at row 1 of the padded plane
NT = 26          # 26 tiles of 128 px cover padded rows 1..56
NCORES = 8

_cache = {}


def _delta(ij):
    i, j = divmod(ij, 3)
    return (i - 1) * WP + (j - 1)


def _build(use_alpha: int):
    f32 = mybir.dt.float32
    bf16 = mybir.dt.bfloat16

    nc = bacc.Bacc("TRN2", target_bir_lowering=False, debug=False,
                   num_devices=NCORES)

    x_d = nc.dram_tensor("x", [C, H, W], f32, kind="ExternalInput")
    alpha_d = nc.dram_tensor("alpha", [G, H, W], f32, kind="ExternalInput")
    tmpl_d = nc.dram_tensor("tmpl", [9, C, T * O], bf16, kind="ExternalInput")
    rwt_d = nc.dram_tensor("rwt", [C, G * T], f32, kind="ExternalInput")
    rb_d = nc.dram_tensor("rb", [G * T], f32, kind="ExternalInput")
    bias_d = nc.dram_tensor("bias", [O], f32, kind="ExternalInput")
    mask_d = None
    if not use_alpha:
        mask_d = nc.dram_tensor("mask", [H, W], mybir.dt.int32,
                                kind="ExternalInput")
    out_d = nc.dram_tensor("out", [O, OFREE], f32, kind="ExternalOutput")

    with TileContext(nc) as tc:
        with (
            tc.tile_pool(name="big", bufs=1) as big,
            tc.tile_pool(name="consts", bufs=1) as consts,
            tc.tile_pool(name="stage", bufs=3) as stage,
            tc.tile_pool(name="acc", bufs=3) as accp,
            tc.tile_pool(name="upool", bufs=3) as upool,
            tc.tile_pool(name="zps", bufs=2, space="PSUM") as zps,
            tc.tile_pool(name="ups", bufs=2, space="PSUM") as ups,
            tc.tile_pool(name="tps", bufs=2, space="PSUM") as tps,
        ):
            # ---- constants ----
            ident = consts.tile([128, 128], f32)
            masks.make_identity(nc, ident[:])

            # PE warmup: ~4us of dummy matmuls so HAM un-throttles before
            # the real work arrives (it only needs SBUF-resident data)
            warm = tps.tile([128, 128], f32, tag="tp", name="warm")
            for w_i in range(32):
                nc.tensor.matmul(warm[:], lhsT=ident[:], rhs=ident[:])

            bias_rep = consts.tile([128, O], f32)
            nc.sync.dma_start(
                out=bias_rep[:],
                in_=bass.AP(tensor=bias_d, offset=0, ap=[[0, 128], [1, O]]),
            )

            # ---- image: two contiguous DMAs + on-chip pad-insert/casts,
            # so the first pixel tiles only wait for the first half ----
            # half A: pixel tiles k=0..12 read pf [-1, 1781); image rows 0..29
            # half B: k=13..25 read pf [1663, 3445); image rows 27..55
            xst0 = big.tile([C, 30 * W], f32)
            nc.sync.dma_start(out=xst0[:], in_=x_d[:, 0:30, :])

            XB1 = 27 * WP                      # pf origin of half B buffer
            xbf0 = big.tile([C, GUARD + 31 * WP], bf16)   # pf [0, 1798)
            nc.gpsimd.memset(xbf0[:], 0.0)
            v = xbf0[:, GUARD:GUARD + 31 * WP].rearrange(
                "c (h w) -> c h w", w=WP)
            nc.vector.tensor_copy(v[:, 1:31, 1:57], xst0[:])

            # ---- templates: bf16, DMA straight in per offset ----
            tbf = []
            for ij in range(9):
                tb = big.tile([C, T * O], bf16, name=f"tbf{ij}")
                nc.sync.dma_start(out=tb[:], in_=tmpl_d[ij])
                tbf.append(tb)

            xst1 = big.tile([C, 29 * W], f32)
            nc.sync.dma_start(out=xst1[:], in_=x_d[:, 27:56, :])
            xbf1 = big.tile([C, 33 * WP], bf16)           # pf [1566, 3480)
            nc.gpsimd.memset(xbf1[:], 0.0)
            v = xbf1[:, 0:33 * WP].rearrange("c (h w) -> c h w", w=WP)
            nc.vector.tensor_copy(v[:, 1:30, 1:57], xst1[:])

            # ---- output accumulation plane, 4 window-aligned chunks so
            # stores overlap compute and the tail only waits on the last ----
            OCUT = [0, PT0 + 128 * 7, PT0 + 128 * 13, PT0 + 128 * 20, OFREE]
            outsb = [big.tile([O, OCUT[i + 1] - OCUT[i]], f32,
                              name=f"outsb{i}") for i in range(4)]

            def outsb_slice(lo, n):
                for i in range(4):
                    if lo < OCUT[i + 1]:
                        assert lo + n <= OCUT[i + 1]
                        return outsb[i][:, lo - OCUT[i]:lo - OCUT[i] + n]
                raise AssertionError(lo)

            # ---- routing: GAP -> fc -> sigmoid ----
            xsum = consts.tile([C, 1], f32)
            xsum0 = consts.tile([C, 1], f32)
            nc.vector.tensor_reduce(
                out=xsum0[:], in_=xst0[:],
                axis=mybir.AxisListType.X, op=mybir.AluOpType.add)
            nc.vector.tensor_reduce(
                out=xsum[:], in_=xst1[:, 3 * W:],
                axis=mybir.AxisListType.X, op=mybir.AluOpType.add)
            nc.vector.tensor_add(xsum[:], xsum[:], xsum0[:])

            rwt = consts.tile([C, G * T], f32)
            nc.sync.dma_start(out=rwt[:], in_=rwt_d[:])
            rb = consts.tile([G * T, 1], f32)
            nc.sync.dma_start(out=rb[:], in_=rb_d[:])

            zr = ups.tile([G * T, 1], f32, tag="up")
            nc.tensor.matmul(zr[:], lhsT=rwt[:], rhs=xsum[:])
            # x_se = (2/T)*sigmoid(fc(mean) + rb); mean folded into scale
            xse = consts.tile([G * T, 1], f32)
            nc.scalar.activation(xse[:], zr[:],
                                 mybir.ActivationFunctionType.Sigmoid,
                                 bias=rb[:], scale=1.0 / (H * W))
            xse4 = consts.tile([G * T, 1], f32)
            nc.vector.tensor_scalar_mul(xse4[:], xse[:], 2.0 / T)

            # lhsT_U [g, T+1]: cols 0..T-1 = x_se[g, t], col T = 1.0
            # (the [64,1] -> [8,8] partition/free reshape is a tiny DMA)
            lhsu = consts.tile([G, T + 1], f32)
            nc.vector.memset(lhsu[:, T:T + 1], 1.0)
            nc.sync.dma_start(out=lhsu[:, 0:T], in_=xse4[:])

            # ---- routing probability numerators ----
            ea = big.tile([G, OFREE], f32)
            nc.gpsimd.memset(ea[:], 1.0)
            ea_core = ea[:, 0:NPIX].rearrange("g (h w) -> g h w", w=WP)
            if use_alpha:
                astage = stage.tile([G, H * W], f32, tag="astage")
                nc.sync.dma_start(out=astage[:], in_=alpha_d[:])
                nc.scalar.activation(ea_core[:, 1:57, 1:57], astage[:],
                                     mybir.ActivationFunctionType.Exp)
            else:
                # hard routing: ea[g, p] = (mask[p] == g)
                mrow = stage.tile([1, OFREE], mybir.dt.int32, tag="mrow")
                nc.vector.memset(mrow[:], 0)
                mrow_core = mrow[:, 0:NPIX].rearrange("a (h w) -> a h w", w=WP)
                nc.sync.dma_start(out=mrow_core[:, 1:57, 1:57],
                                  in_=mask_d[:])
                mf = stage.tile([1, OFREE], f32, tag="mf")
                nc.scalar.copy(mf[:], mrow[:])
                mrep = big.tile([G, OFREE], f32)
                for g in range(G):
                    nc.sync.dma_start(out=mrep[g:g + 1, :], in_=mf[:])
                giota = consts.tile([G, 1], f32)
                for g in range(G):
                    nc.vector.memset(giota[g:g + 1, :], float(g))
                nc.vector.tensor_scalar(ea[:], mrep[:], giota[:], None,
                                        op0=mybir.AluOpType.is_equal)

            # ---- main loop over pixel tiles ----
            for k in range(NT):
                base = PT0 + 128 * k

                up = ups.tile([128, T + 1], f32, tag="up")
                nc.tensor.matmul(up[:], lhsT=ea[:, base:base + 128],
                                 rhs=lhsu[:])
                rcol = upool.tile([128, 1], f32, tag="rcol")
                nc.vector.reciprocal(rcol[:], up[:, T:T + 1])
                usb = upool.tile([128, T], f32, tag="usb")
                nc.vector.tensor_scalar_mul(usb[:], up[:, 0:T], rcol[:])

                zp = [zps.tile([128, 512], f32, tag=f"zp{h}",
                               name=f"zp{h}_{k}")
                      for h in range(2)]
                for ij in range(9):
                    if k <= 12:
                        lo = GUARD + base + _delta(ij)
                        xsl = xbf0[:, lo:lo + 128]
                    else:
                        lo = base - XB1 + _delta(ij)
                        xsl = xbf1[:, lo:lo + 128]
                    for h in range(2):
                        nc.tensor.matmul(
                            zp[h][:],
                            lhsT=xsl,
                            rhs=tbf[ij][:, h * 512:(h + 1) * 512],
                            start=(ij == 0), stop=(ij == 8))

                acc = accp.tile([128, O], f32, tag="acc")
                for t in range(T):
                    h, tq = divmod(t, 4)
                    nc.vector.scalar_tensor_tensor(
                        out=acc[:],
                        in0=zp[h][:, tq * 128:(tq + 1) * 128],
                        scalar=usb[:, t:t + 1],
                        in1=bias_rep[:] if t == 0 else acc[:],
                        op0=mybir.AluOpType.mult,
                        op1=mybir.AluOpType.add)

                tp = tps.tile([128, 128], f32, tag="tp")
                nc.tensor.transpose(tp[:], acc[:], ident[:])
                nc.scalar.copy(outsb_slice(base, 128), tp[:])

            # ---- store padded planes (host strips the padding) ----
            for i in range(4):
                nc.sync.dma_start(out=out_d[:, OCUT[i]:OCUT[i + 1]],
                                  in_=outsb[i][:])

    nc.compile()
    return nc


def _get(use_alpha: int):
    if use_alpha not in _cache:
        _cache[use_alpha] = _build(use_alpha)
    return _cache[use_alpha]


def _in_maps(inp):
    ua = int(np.asarray(inp["use_alpha"]))
    inputs = np.ascontiguousarray(np.asarray(inp["inputs"], dtype=np.float32))
    Alpha = np.ascontiguousarray(np.asarray(inp["Alpha"], dtype=np.float32))
    # [O*C*3*3, T] -> [(i,j), c, t*O + o]
    tmpl = np.asarray(inp["weight_templates"], dtype=np.float32).reshape(
        O, C, 3, 3, T).transpose(2, 3, 1, 4, 0).reshape(9, C, T * O)
    tmpl = np.ascontiguousarray(tmpl).astype(ml_dtypes.bfloat16)
    rwt = np.ascontiguousarray(
        np.asarray(inp["routing_w"], dtype=np.float32).T)
    rb = np.ascontiguousarray(np.asarray(inp["routing_b"], dtype=np.float32))
    bias = np.ascontiguousarray(np.asarray(inp["bias"], dtype=np.float32))

    in_maps = []
    for b in range(NCORES):
        m = {"x": inputs[b], "alpha": Alpha[b], "tmpl": tmpl, "rwt": rwt,
             "rb": rb, "bias": bias}
        if not ua:
            m["mask"] = np.ascontiguousarray(
                np.asarray(inp["mask"][b], dtype=np.int32))
        in_maps.append(m)
    return in_maps


def kernel(inputs, mask, Alpha, weight_templates, routing_w, routing_b, bias,
           use_alpha):
    ua = int(np.asarray(use_alpha))
    nc = _get(ua)
    in_maps = _in_maps(dict(inputs=inputs, mask=mask, Alpha=Alpha,
                            weight_templates=weight_templates,
                            routing_w=routing_w, routing_b=routing_b,
                            bias=bias, use_alpha=use_alpha))
    res = run_bass_kernel_spmd(nc, in_maps, list(range(NCORES)))
    out = np.stack([res.results[b]["out"] for b in range(NCORES)], axis=0)
    out = out[:, :, :NPIX].reshape(NCORES, O, HP, WP)[:, :, 1:57, 1:57]
    return np.ascontiguousarray(out)


# revision 13
# speedup vs baseline: 1.0511x; 1.0511x over previous
"""DRConv (dynamic region-aware conv) Trainium2 kernel.

Math (per batch b, all on device):
  x_se  = 0.25*sigmoid(routing_w @ mean_hw(x) + routing_b)           # [G*T]
  Z_t   = conv3x3(x, template_t)       for t in 0..T-1               # [O, H, W]
  U     = [x_se.T | 1] contracted with exp(Alpha) over g             # [T+1, P]
  out   = (sum_t Z_t * U_t) / U_T  + bias                            # [O, H, W]
which equals the reference
  out = einsum('boghw,bghw->bohw', einsum('bokg,bkhw->boghw', w, patches),
               softmax(Alpha)) + bias
because w = blend(x_se, templates) commutes through the conv: the blend
weights x_se[g,t] and the softmax probs both act per (g, pixel), so the
G-sum and T-sum exchange with the K-contraction.

Sharding: data-parallel over batch B=8, one batch element per NeuronCore.
Templates/routing weights replicated. No collectives.

Device layout (per core):
  pixels live in a 58x57 plane: one pad row top/bottom, ONE pad column
  (a right-pad column doubles as the left neighbor of the next row's
  x=0 pixel, so 57-wide rows give correct 3x3 zero padding);
  pf = (y+1)*57 + x for image pixel (y, x).
  conv = 9 shifted matmuls accumulating in PSUM:
    Z[px, (t,o)] += x[c, base+px+delta(i,j)].T @ tmpl[c, (t,o)]
  pixel tiles are the stationary operand (128 px per matmul), so the
  per-pixel softmax mixing becomes per-partition scalar_tensor_tensor ops,
  and the final [px, o] -> [o, px] flip is a PE transpose.
"""

import ml_dtypes
import numpy as np

import concourse.bass as bass
import concourse.mybir as mybir
from concourse import bacc, masks
from concourse.tile import TileContext
from concourse.bass_utils import run_bass_kernel_spmd

# problem constants
C = 128          # in channels
O = 128          # out channels
H = W = 56
G = 8            # groups
T = 8            # num weight templates
WP = 57          # padded row width (one shared pad column)
HPAD = 58        # one pad row top and bottom
NPIX = HPAD * WP  # 3306
GUARD = 64       # front guard in the x buffer for negative conv shifts
OFREE = 3328     # 26*128 >= NPIX
PT0 = WP         # first pixel-tile starts at padded row 1
NT = 25          # 25 tiles of 128 px cover pf [57, 3257) > last valid 3247
NCORES = 8

_cache = {}


def _delta(ij):
    i, j = divmod(ij, 3)
    return (i - 1) * WP + (j - 1)


def _build(use_alpha: int):
    f32 = mybir.dt.float32
    bf16 = mybir.dt.bfloat16

    nc = bacc.Bacc("TRN2", target_bir_lowering=False, debug=False,
                   num_devices=NCORES)

    # image ships as bf16 (matmuls are bf16 anyway), split in two row
    # bands so early pixel tiles only wait for the first band
    x0_d = nc.dram_tensor("x0", [C, 31 * W], bf16, kind="ExternalInput")
    x1_d = nc.dram_tensor("x1", [C, 28 * W], bf16, kind="ExternalInput")
    alpha_d = nc.dram_tensor("alpha", [G, H, W], f32, kind="ExternalInput")
    tmpl_d = nc.dram_tensor("tmpl", [9, C, T * O], bf16, kind="ExternalInput")
    rwt_d = nc.dram_tensor("rwt", [C, G * T], f32, kind="ExternalInput")
    rb_d = nc.dram_tensor("rb", [G * T], f32, kind="ExternalInput")
    bias_d = nc.dram_tensor("bias", [O], f32, kind="ExternalInput")
    mask_d = None
    if not use_alpha:
        mask_d = nc.dram_tensor("mask", [H, W], mybir.dt.int32,
                                kind="ExternalInput")
    out_d = nc.dram_tensor("out", [O, OFREE], f32, kind="ExternalOutput")

    with TileContext(nc) as tc:
        with (
            tc.tile_pool(name="big", bufs=1) as big,
            tc.tile_pool(name="consts", bufs=1) as consts,
            tc.tile_pool(name="stage", bufs=3) as stage,
            tc.tile_pool(name="acc", bufs=3) as accp,
            tc.tile_pool(name="upool", bufs=3) as upool,
            tc.tile_pool(name="zps", bufs=2, space="PSUM") as zps,
            tc.tile_pool(name="ups", bufs=2, space="PSUM") as ups,
            tc.tile_pool(name="tps", bufs=2, space="PSUM") as tps,
        ):
            # ---- constants ----
            ident = consts.tile([128, 128], f32)
            masks.make_identity(nc, ident[:])

            # PE warmup: dummy matmuls so HAM un-throttles while the
            # input DMAs stream in (needs only SBUF-resident data)
            warm = tps.tile([128, 128], f32, tag="tp", name="warm")
            for w_i in range(28):
                nc.tensor.matmul(warm[:], lhsT=ident[:], rhs=ident[:])

            bias_rep = consts.tile([128, O], f32)
            nc.sync.dma_start(
                out=bias_rep[:],
                in_=bass.AP(tensor=bias_d, offset=0, ap=[[0, 128], [1, O]]),
            )

            # ---- image band A + routing weights first ----
            xst0 = big.tile([C, 31 * W], bf16)
            nc.sync.dma_start(out=xst0[:], in_=x0_d[:])
            rwt = consts.tile([C, G * T], f32)
            nc.sync.dma_start(out=rwt[:], in_=rwt_d[:])
            rb = consts.tile([G * T, 1], f32)
            nc.sync.dma_start(out=rb[:], in_=rb_d[:])

            # pixel tiles k<=12 read pf [-1, 1779) -> image rows 0..30
            xbf0 = big.tile([C, GUARD + 32 * WP], bf16)
            nc.gpsimd.memset(xbf0[:], 0.0)
            v = xbf0[:, GUARD:GUARD + 32 * WP].rearrange(
                "c (h w) -> c h w", w=WP)
            nc.gpsimd.tensor_copy(
                v[:, 1:32, 0:W], xst0[:].rearrange("c (h w) -> c h w", w=W))

            # ---- templates ----
            tbf = []
            for ij in range(9):
                tb = big.tile([C, T * O], bf16, name=f"tbf{ij}")
                nc.sync.dma_start(out=tb[:], in_=tmpl_d[ij])
                tbf.append(tb)

            # ---- image band B: k>=13 read pf [1663, 3315); rows 28..55 ----
            XB1 = 29 * WP                  # pf origin of band B buffer
            xst1 = big.tile([C, 28 * W], bf16)
            nc.sync.dma_start(out=xst1[:], in_=x1_d[:])
            xbf1 = big.tile([C, 30 * WP], bf16)
            nc.gpsimd.memset(xbf1[:], 0.0)
            v = xbf1[:, 0:30 * WP].rearrange("c (h w) -> c h w", w=WP)
            nc.gpsimd.tensor_copy(
                v[:, 0:28, 0:W], xst1[:].rearrange("c (h w) -> c h w", w=W))

            # ---- routing: GAP -> fc -> sigmoid (start ASAP) ----
            xsum = consts.tile([C, 1], f32)
            xsum0 = consts.tile([C, 1], f32)
            nc.vector.tensor_reduce(
                out=xsum0[:], in_=xst0[:],
                axis=mybir.AxisListType.X, op=mybir.AluOpType.add)
            nc.vector.tensor_reduce(
                out=xsum[:], in_=xst1[:, 3 * W:],
                axis=mybir.AxisListType.X, op=mybir.AluOpType.add)
            nc.vector.tensor_add(xsum[:], xsum[:], xsum0[:])

            zr = ups.tile([G * T, 1], f32, tag="up")
            nc.tensor.matmul(zr[:], lhsT=rwt[:], rhs=xsum[:])
            # x_se = (2/T)*sigmoid(fc(mean) + rb); mean folded into scale
            xse = consts.tile([G * T, 1], f32)
            nc.scalar.activation(xse[:], zr[:],
                                 mybir.ActivationFunctionType.Sigmoid,
                                 bias=rb[:], scale=1.0 / (H * W))
            xse4 = consts.tile([G * T, 1], f32)
            nc.vector.tensor_scalar_mul(xse4[:], xse[:], 2.0 / T)

            # lhsT_U [g, T+1]: cols 0..T-1 = x_se[g, t], col T = 1.0
            # (the [64,1] -> [8,8] partition/free reshape is a tiny DMA)
            lhsu = consts.tile([G, T + 1], f32)
            nc.vector.memset(lhsu[:, T:T + 1], 1.0)
            nc.sync.dma_start(out=lhsu[:, 0:T], in_=xse4[:])

            # ---- routing probability numerators ----
            ea = big.tile([G, OFREE], f32)
            nc.gpsimd.memset(ea[:], 1.0)
            ea_core = ea[:, 0:NPIX].rearrange("g (h w) -> g h w", w=WP)
            if use_alpha:
                astage = stage.tile([G, H * W], f32, tag="astage")
                nc.sync.dma_start(out=astage[:], in_=alpha_d[:])
                nc.scalar.activation(
                    ea_core[:, 1:57, 0:W],
                    astage[:].rearrange("g (h w) -> g h w", w=W),
                    mybir.ActivationFunctionType.Exp)
            else:
                # hard routing: ea[g, p] = (mask[p] == g)
                mrow = stage.tile([1, H * W], mybir.dt.int32, tag="mrow")
                nc.sync.dma_start(out=mrow[:], in_=mask_d[:])
                mf = stage.tile([1, H * W], f32, tag="mf")
                nc.scalar.copy(mf[:], mrow[:])
                mrep = big.tile([G, H * W], f32)
                for g in range(G):
                    nc.sync.dma_start(out=mrep[g:g + 1, :], in_=mf[:])
                giota = consts.tile([G, 1], f32)
                for g in range(G):
                    nc.vector.memset(giota[g:g + 1, :], float(g))
                nc.vector.tensor_scalar(
                    ea_core[:, 1:57, 0:W],
                    mrep[:].rearrange("g (h w) -> g h w", w=W),
                    giota[:], None, op0=mybir.AluOpType.is_equal)

            # ---- output accumulation plane, 4 window-aligned chunks so
            # stores overlap compute and the tail only waits on the last ----
            OCUT = [0, PT0 + 128 * 7, PT0 + 128 * 13, PT0 + 128 * 19, OFREE]
            outsb = [big.tile([O, OCUT[i + 1] - OCUT[i]], f32,
                              name=f"outsb{i}") for i in range(4)]

            def outsb_slice(lo, n):
                for i in range(4):
                    if lo + n <= OCUT[i + 1]:
                        assert lo >= OCUT[i]
                        return outsb[i][:, lo - OCUT[i]:lo - OCUT[i] + n]
                raise AssertionError(lo)

            # ---- main loop over pixel tiles ----
            for k in range(NT):
                base = PT0 + 128 * k

                up = ups.tile([128, T + 1], f32, tag="up")
                nc.tensor.matmul(up[:], lhsT=ea[:, base:base + 128],
                                 rhs=lhsu[:])
                rcol = upool.tile([128, 1], f32, tag="rcol")
                nc.vector.reciprocal(rcol[:], up[:, T:T + 1])
                usb = upool.tile([128, T], f32, tag="usb")
                nc.vector.tensor_scalar_mul(usb[:], up[:, 0:T], rcol[:])

                zp = [zps.tile([128, 512], f32, tag=f"zp{h}",
                               name=f"zp{h}_{k}")
                      for h in range(2)]
                for ij in range(9):
                    if k <= 12:
                        lo = GUARD + base + _delta(ij)
                        xsl = xbf0[:, lo:lo + 128]
                    else:
                        lo = base - XB1 + _delta(ij)
                        xsl = xbf1[:, lo:lo + 128]
                    for h in range(2):
                        nc.tensor.matmul(
                            zp[h][:],
                            lhsT=xsl,
                            rhs=tbf[ij][:, h * 512:(h + 1) * 512],
                            start=(ij == 0), stop=(ij == 8))

                acc = accp.tile([128, O], f32, tag="acc")
                for t in range(T):
                    h, tq = divmod(t, 4)
                    nc.vector.scalar_tensor_tensor(
                        out=acc[:],
                        in0=zp[h][:, tq * 128:(tq + 1) * 128],
                        scalar=usb[:, t:t + 1],
                        in1=bias_rep[:] if t == 0 else acc[:],
                        op0=mybir.AluOpType.mult,
                        op1=mybir.AluOpType.add)

                tp = tps.tile([128, 128], f32, tag="tp")
                nc.tensor.transpose(tp[:], acc[:], ident[:])
                nc.scalar.copy(outsb_slice(base, 128), tp[:])

            # ---- store padded planes (host strips the padding) ----
            for i in range(4):
                nc.sync.dma_start(out=out_d[:, OCUT[i]:OCUT[i + 1]],
                                  in_=outsb[i][:])

    nc.compile()
    return nc


def _get(use_alpha: int):
    if use_alpha not in _cache:
        _cache[use_alpha] = _build(use_alpha)
    return _cache[use_alpha]


def _in_maps(inp):
    ua = int(np.asarray(inp["use_alpha"]))
    x = np.asarray(inp["inputs"], dtype=np.float32).reshape(
        NCORES, C, H * W).astype(ml_dtypes.bfloat16)
    x0 = np.ascontiguousarray(x[:, :, 0:31 * W])
    x1 = np.ascontiguousarray(x[:, :, 28 * W:])
    Alpha = np.ascontiguousarray(np.asarray(inp["Alpha"], dtype=np.float32))
    # [O*C*3*3, T] -> [(i,j), c, t*O + o]
    tmpl = np.asarray(inp["weight_templates"], dtype=np.float32).reshape(
        O, C, 3, 3, T).transpose(2, 3, 1, 4, 0).reshape(9, C, T * O)
    tmpl = np.ascontiguousarray(tmpl).astype(ml_dtypes.bfloat16)
    rwt = np.ascontiguousarray(
        np.asarray(inp["routing_w"], dtype=np.float32).T)
    rb = np.ascontiguousarray(np.asarray(inp["routing_b"], dtype=np.float32))
    bias = np.ascontiguousarray(np.asarray(inp["bias"], dtype=np.float32))

    in_maps = []
    for b in range(NCORES):
        m = {"x0": x0[b], "x1": x1[b], "alpha": Alpha[b], "tmpl": tmpl,
             "rwt": rwt, "rb": rb, "bias": bias}
        if not ua:
            m["mask"] = np.ascontiguousarray(
                np.asarray(inp["mask"][b], dtype=np.int32))
        in_maps.append(m)
    return in_maps


def kernel(inputs, mask, Alpha, weight_templates, routing_w, routing_b, bias,
           use_alpha):
    ua = int(np.asarray(use_alpha))
    nc = _get(ua)
    in_maps = _in_maps(dict(inputs=inputs, mask=mask, Alpha=Alpha,
                            weight_templates=weight_templates,
                            routing_w=routing_w, routing_b=routing_b,
                            bias=bias, use_alpha=use_alpha))
    res = run_bass_kernel_spmd(nc, in_maps, list(range(NCORES)))
    out = np.stack([res.results[b]["out"] for b in range(NCORES)], axis=0)
    out = out[:, :, :NPIX].reshape(NCORES, O, HPAD, WP)[:, :, 1:57, 0:W]
    return np.ascontiguousarray(out)
